# revision 34
# baseline (speedup 1.0000x reference)
"""Trainium2 Bass kernel for nn_DualAttention (S=2048, B=16, H2=2048, V=1024).

Computation (per the reference):
    sum_w = hidden @ Ww + bw + z @ Wz + bz + w_a*0.5        [S, B, V]
    u     = tanh(sum_w) @ Vw + vb                            [S, B, 1]
    out   = softmax(u, axis=0)                               [S, B, 1]

Strategy
--------
Data-parallel over batch: 16 batches -> 2 per NeuronCore (8 cores).
Host-side prep per core:
  * concat hidden/z along the hidden axis -> X [ROWS=4096, H=4096]
    (rows are b-major: row = b_local*2048 + s)
  * transpose + pack X^T into [NRB, P, NK, RB] (per-partition contiguous
    chunks for each (rowblock, k-group) DMA), cast to bf16
  * W = concat([Ww, Wz], 0) [H, V], reordered into per-(vb,k) 128x128
    tiles; bias = bw + bz + 0.5*w_a
Device kernel (per core), W-stationary matmul with psum layout [v, rows]:
  ~26 warmup matmuls on zeroed scratch at t=0 (HAM clock ramp overlaps
  the initial DMA wait), then for each rowblock (RB=512 rows):
    for vb in 0..7:                       # 128-wide slices of V
      psum[vb] += sum_k W[vb,k].T @ xt[k]      (32 accumulating matmuls)
      t = tanh(psum + bias_vb)            # one ACT op, per-partition bias
      s_acc = (t * vwt[:,vb]) + s_acc     # fused DVE op (second stage)
    u_psum = ones.T @ s_acc               # one PE partition-reduction
    att[rowblock] = exp(u_psum)           # ACT reads psum, DMA out
  (the reduction/exp for rowblock r is emitted one mm-group into
  rowblock r+1 so the in-order PE never waits on ACT/DVE)
The device emits exp(u); the softmax normalization (one scalar divide
per batch) and the final gather happen host-side.

The vb scalar is dropped: softmax is shift-invariant.

MAIN_DT selects the matmul dtype: "bf16" (PE roofline 216ns/MM @N=512,
~1e-2 rel err) or "f32r" (fp32 data, +13.5ns/MM fixed overhead,
~1e-3 rel err). Measured on HW: bf16 N=512 back-to-back spacing 216ns
(ideal 213.3), f32r N=256 120ns (ideal 106.7).
"""

import numpy as np
import ml_dtypes

# ---------------------------------------------------------------------------
# Problem constants (hardcoded; kernel.py must be self-contained)
# ---------------------------------------------------------------------------
S, B, H2, V = 2048, 16, 2048, 1024
ALPHA_S = 0.5
NCORES = 8
BC = B // NCORES            # local batches per core
ROWS = S * BC               # 4096 rows per core (b-major)
H = 2 * H2                  # 4096 contraction dim (hidden ++ z)
P = 128
NK = H // P                 # 32
NVB = V // P                # 8

MAIN_DT = "bf16"            # "bf16" | "f32r"
RB = 512 if MAIN_DT == "bf16" else 256
NRB = ROWS // RB
# 8 = 2 rowblocks of 4 k-group tiles: rowblock r+2's prefetch DMAs reuse
# rowblock r's buffers, so the sync engine's issue of group g naturally
# waits until group g of rowblock r is consumed — staggering the prefetch
# traffic evenly across the rowblock instead of bursting it all at once.
XT_BUFS = 8 if MAIN_DT == "bf16" else 7
RPB = NRB // BC             # rowblocks per local batch


# ---------------------------------------------------------------------------
# Workarounds for this walrus build's 1-sync-wait-per-instruction limit
# ---------------------------------------------------------------------------
def _install_drain_patch():
    import concourse.mybir as mybir
    from concourse.tile import TileContext
    from concourse.vector_clock import ScopedClock

    def _drain_and_barrier(self, tick_clock, wait_clock):
        nc = self.nc
        drain_inst = nc.sync.drain()
        wait_clock.add_sem_waits(
            drain_inst.ins, ScopedClock({None: tick_clock.global_clock})
        )
        si = drain_inst.ins.sync_info
        if si is not None:
            waits = list(si.on_wait)
            if len(waits) > 1:
                si.on_wait = [waits[0]]
                for w in waits[1:]:
                    nop = nc.sync.nop(nofuse=True)
                    nop.ins.sync_info = mybir.SyncInfo(on_wait=[w], on_update=[])
        nc.all_engine_barrier()
        assert self.sems is not None
        popped = nc._tile_sem_poison_stack.pop()
        assert popped is self._sem_poison
        nc.clear_and_free_semaphores(list(self.sems.allocated().values()))
        nc.all_engine_barrier()

    TileContext._drain_and_barrier = _drain_and_barrier


def _split_multiwait(nc):
    """Hoist extra sync waits onto same-engine event-semaphore instructions
    inserted just before the carrying instruction."""
    import concourse.mybir as mybir

    counter = 0
    for fn in nc.m.functions:
        for bb in fn.blocks:
            insts = bb.instructions
            new_list = []
            changed = False
            for inst in insts:
                si = inst.sync_info
                if si is not None:
                    waits = list(si.on_wait)
                    if len(waits) > 1:
                        for w in waits[:-1]:
                            counter += 1
                            nop = mybir.InstEventSemaphore(
                                name=f"I-mwsplit-{counter}"
                            )
                            nop.engine = inst.engine
                            nop.bass_nofuse = True
                            nop.sync_info = mybir.SyncInfo(
                                on_wait=[w], on_update=[]
                            )
                            nc.register_instruction(nop)
                            new_list.append(nop)
                        si.on_wait = [waits[-1]]
                        changed = True
                new_list.append(inst)
            if changed:
                bb.instructions = new_list
    return counter


# ---------------------------------------------------------------------------
# Kernel build
# ---------------------------------------------------------------------------
def _build_nc():
    import concourse.bass as bass
    import concourse.mybir as mybir
    from concourse.tile import TileContext

    f32 = mybir.dt.float32
    f32r = mybir.dt.float32r
    DT = mybir.dt.bfloat16 if MAIN_DT == "bf16" else f32r

    nc = bass.Bass()
    # W pre-tiled host-side: tile (vb, k) is [P, 128] contiguous
    w_d = nc.declare_dram_parameter("w", [NVB, P, NK * P], DT, isOutput=False)
    # X^T packed host-side: xt[r, p, k, c] = X^T[k*P+p, r*RB+c]
    xt_d = nc.declare_dram_parameter("xt", [NRB, P, NK, RB], DT, isOutput=False)
    bct_d = nc.declare_dram_parameter("bct", [P, NVB], f32, isOutput=False)
    vwt_d = nc.declare_dram_parameter("vwt", [P, NVB], f32, isOutput=False)
    ones_d = nc.declare_dram_parameter("ones", [P, 1], f32r, isOutput=False)
    # vwt's last column as f32r: stationary operand of the fused vb7
    # reduction matmul in the final rowblock's tail
    vwt7_d = nc.declare_dram_parameter("vwt7", [P, 1], f32r, isOutput=False)
    # att holds exp(u); the softmax normalization (one scalar divide per
    # batch) happens host-side on the gathered output
    att_d = nc.declare_dram_parameter("att", [BC, S], f32, isOutput=True)

    with TileContext(nc) as tc:
        with (
            tc.tile_pool(name="wpool", bufs=1) as wpool,
            tc.tile_pool(name="xpool", bufs=1) as xpool,
            tc.tile_pool(name="tpool", bufs=1) as tpool,
            tc.tile_pool(name="spool", bufs=1) as spool,
            tc.tile_pool(name="pspool", bufs=1, space="PSUM") as pspool,
        ):
            # --- constants: allocated here, issued at the END of the head
            # stream (they are not consumed until the first tanh ~55us in,
            # and must not delay the latency-critical first xt/w chunks)
            bct_sb = spool.tile([P, NVB], f32, name="bct_sb")
            vwt_sb = spool.tile([P, NVB], f32, name="vwt_sb")
            ones_sb = spool.tile([P, 1], f32r, name="ones_sb")
            vwt7_sb = spool.tile([P, 1], f32r, name="vwt7_sb")

            # --- HAM warmup: the PE clock sits at 1.2 GHz until ~3.4us of
            # sustained matmul activity. Run short (N=128) throwaway
            # matmuls on a small zeroed tile during the initial DMA wait so
            # the ramp completes while the first xt/w chunks are in flight;
            # short matmuls make the warmup->real handoff prompt.
            wu_x = spool.tile([P, P], DT, name="wu_x")
            nc.vector.memset(wu_x[:], 0.0)
            wu_ps = pspool.tile([P, RB], f32, name="wu_ps", tag="ps", bufs=8)
            for _ in range(46):
                nc.tensor.matmul(
                    wu_ps[:, 0:P], wu_x[:], wu_x[:], start=True, stop=True
                )

            # --- resident weights + rowblock-0 xt, co-scheduled
            # k-progressively. Rowblock 0 runs with all 8 vb interleaved
            # (k-major), consuming one k-tile per 8 matmuls (1.73us):
            # that needs xt at ~74 GB/s and W at ~148 GB/s, together right
            # at the achievable aggregate (~220 GB/s over the queue set),
            # so the issue order delivers W chunks for ALL vb in lockstep
            # with the xt k-groups. Single-issuer (sync) on purpose: a
            # second issuing engine interleaves badly in the shared queue
            # FIFOs (measured 492/522us vs the 483us band).
            # One W tile for all vb so a single DMA can span a vb-PAIR:
            # the early k0-1 chunks for all 8 vb then take 4 issues
            # instead of 8 (the sync ring's ~600ns/issue is the head's
            # scarcest resource).
            w_all = wpool.tile([P, NVB, NK, P], DT, name="w_all")

            def w_chunk(vb, k0, k1):
                nc.sync.dma_start(
                    out=w_all[:, vb, k0:k1],
                    in_=w_d[vb, :, k0 * P : k1 * P].rearrange(
                        "p (k q) -> p k q", q=P
                    ),
                )

            def w_pair_chunk(v0, k0, k1):
                nc.sync.dma_start(
                    out=w_all[:, v0 : v0 + 2, k0:k1],
                    in_=w_d[v0 : v0 + 2, :, k0 * P : k1 * P].rearrange(
                        "v p (k q) -> p v k q", q=P
                    ),
                )

            KG = 8
            NKG = NK // KG

            def load_xt(r, xsplit=2, eng=None):
                issuer = eng if eng is not None else nc.sync
                tiles = []
                for g in range(NKG):
                    t = xpool.tile(
                        [P, KG, RB], DT, name=f"xt_{r}_{g}", tag="xt",
                        bufs=XT_BUFS,
                    )
                    kc = KG // xsplit
                    for j in range(xsplit):
                        issuer.dma_start(
                            out=t[:, j * kc : (j + 1) * kc],
                            in_=xt_d[
                                r, :, g * KG + j * kc : g * KG + (j + 1) * kc, :
                            ],
                        )
                    tiles.append(t)
                return tiles

            xt_tiles = [
                xpool.tile(
                    [P, KG, RB], DT, name=f"xt_0_{g}", tag="xt", bufs=XT_BUFS
                )
                for g in range(NKG)
            ]

            def xt0_chunk(g, j0, j1):
                nc.sync.dma_start(
                    out=xt_tiles[g][:, j0:j1],
                    in_=xt_d[0, :, g * KG + j0 : g * KG + j1, :],
                )

            # Early schedule: xt k0 halves + vb-PAIR k0-1 chunks (4 issues
            # cover all 8 vb), then pair (2,6) chunks, then per-vb chunks
            # sized so each lands just before its k-round deadline.
            nc.sync.dma_start(
                out=xt_tiles[0][:, 0:1, 0 : RB // 2],
                in_=xt_d[0, :, 0:1, 0 : RB // 2],
            )
            w_pair_chunk(0, 0, 2)
            nc.sync.dma_start(
                out=xt_tiles[0][:, 0:1, RB // 2 : RB],
                in_=xt_d[0, :, 0:1, RB // 2 : RB],
            )
            w_pair_chunk(2, 0, 2)
            xt0_chunk(0, 1, 2)
            w_pair_chunk(4, 0, 2)
            w_pair_chunk(6, 0, 2)
            xt0_chunk(0, 2, 3)
            xt0_chunk(0, 3, 4)
            for v0 in (0, 2, 4, 6):
                w_pair_chunk(v0, 2, 6)
            xt0_chunk(0, 4, 6)
            xt0_chunk(0, 6, 8)
            # g1 xt early (4-way split: its k8 deadline is the tightest of
            # the big chunks), interleaved with the per-vb (6,16) W chunks
            for j in range(2):
                xt0_chunk(1, 4 * j, 4 * j + 2)
                w_chunk(4 * j, 6, 16)
                w_chunk(4 * j + 1, 6, 16)
                xt0_chunk(1, 4 * j + 2, 4 * j + 4)
                w_chunk(4 * j + 2, 6, 16)
                w_chunk(4 * j + 3, 6, 16)
            # remainder: per-vb (16,32) W + xt g2/g3
            for j in range(2):
                xt0_chunk(2, 4 * j, 4 * j + 2)
                w_chunk(4 * j, 16, 32)
                w_chunk(4 * j + 1, 16, 32)
                xt0_chunk(2, 4 * j + 2, 4 * j + 4)
                w_chunk(4 * j + 2, 16, 32)
                w_chunk(4 * j + 3, 16, 32)
            xt0_chunk(3, 0, 4)
            xt0_chunk(3, 4, 8)
            # rowblock 1 prefetched behind the rowblock-0/W stream
            xt_next = load_xt(1, xsplit=4) if NRB > 1 else None
            # constants: issued last (on the otherwise-idle scalar ring);
            # first consumer is the first tanh ~55us in
            nc.scalar.dma_start(out=bct_sb[:], in_=bct_d[:, :])
            nc.scalar.dma_start(out=vwt_sb[:], in_=vwt_d[:, :])
            nc.scalar.dma_start(out=ones_sb[:], in_=ones_d[:, :])
            nc.scalar.dma_start(out=vwt7_sb[:], in_=vwt7_d[:, :])

            # Second stage: s_acc[p, c] = sum_vb vwt[p, vb] * tanh_vb[p, c]
            # accumulated on the DVE; one ones-reduction matmul per
            # rowblock turns that into u[c] (partition reduction), and the
            # ACT exp reads that psum directly into SBUF for the output
            # DMA. PE does only 2048 main matmuls + 8 tiny reductions.
            s_acc_of = {}
            u_ps_of = {}

            def epilogue(r):
                """Emitted one mm-group after rowblock r ends: the ones-
                reduction matmul (PE), exp (ACT), and the output DMA.
                u_ps shares the single 8-buf psum rotation (only row 0 of
                the bank is used)."""
                u_ps_of[r] = pspool.tile(
                    [P, RB], f32, name="u_ps", tag="ps", bufs=8
                )
                nc.tensor.matmul(
                    u_ps_of[r][0:1, :],
                    ones_sb[:],
                    s_acc_of[r][:],
                    start=True,
                    stop=True,
                )
                u_att = tpool.tile([1, RB], f32, name="u_att", tag="ua", bufs=4)
                nc.scalar.activation(
                    u_att[:],
                    u_ps_of[r][0:1, :],
                    mybir.ActivationFunctionType.Exp,
                )
                b = r // RPB
                s0 = (r % RPB) * RB
                # issued on the Activation engine: follows the exp in
                # program order, no cross-engine semaphore hop
                nc.scalar.dma_start(
                    out=att_d[b : b + 1, s0 : s0 + RB], in_=u_att[:]
                )

            pending_r = None

            def second_stage(r, vb, ps):
                """tanh + DVE accumulate into s_acc_of[r] for one vb."""
                tt = tpool.tile([P, RB], f32r, name="tt", tag="tt", bufs=3)
                nc.scalar.activation(
                    tt[:],
                    ps[:],
                    mybir.ActivationFunctionType.Tanh,
                    bias=bct_sb[:, vb : vb + 1],
                    scale=1.0,
                )
                if vb == 0:
                    nc.vector.tensor_scalar_mul(
                        s_acc_of[r][:], tt[:], vwt_sb[:, 0:1]
                    )
                else:
                    # s_acc = (tt * vwt[:, vb]) + s_acc, fused on DVE
                    nc.vector.scalar_tensor_tensor(
                        s_acc_of[r][:],
                        tt[:],
                        vwt_sb[:, vb : vb + 1],
                        s_acc_of[r][:],
                        mybir.AluOpType.mult,
                        mybir.AluOpType.add,
                    )

            # --- tail variants for the very last rowblock: vb7's DVE
            # accumulate is replaced by a fused PE reduction (stationary =
            # vwt[:,7] as f32r, moving = tanh output), the second stage and
            # epilogue are split into column halves, and the output DMAs
            # issue from the idle sync ring — so the post-last-matmul
            # serial chain is just tanh(half) -> fused-mm -> exp -> DMA.
            TS = 2
            TC = RB // TS
            tt7_of = {}

            def second_stage_tail_half(r, vb, ps, j):
                if j == 0:
                    tt7_of[r] = tpool.tile(
                        [P, RB], f32r, name="tt", tag="tt", bufs=3
                    )
                sl = slice(j * TC, (j + 1) * TC)
                nc.scalar.activation(
                    tt7_of[r][:, sl],
                    ps[:, sl],
                    mybir.ActivationFunctionType.Tanh,
                    bias=bct_sb[:, vb : vb + 1],
                    scale=1.0,
                )

            def epilogue_tail_half(r, j):
                if j == 0:
                    u_ps_of[r] = pspool.tile(
                        [P, RB], f32, name="u_ps", tag="ps", bufs=8
                    )
                b = r // RPB
                s0 = (r % RPB) * RB
                sl = slice(j * TC, (j + 1) * TC)
                # s_acc holds vb0-6 (ready before the last matmuls);
                # this first mm never stalls the PE
                nc.tensor.matmul(
                    u_ps_of[r][0:1, sl],
                    ones_sb[:],
                    s_acc_of[r][:, sl],
                    start=True,
                    stop=False,
                )
                nc.tensor.matmul(
                    u_ps_of[r][0:1, sl],
                    vwt7_sb[:],
                    tt7_of[r][:, sl],
                    start=False,
                    stop=True,
                )
                u_att = tpool.tile(
                    [1, TC], f32, name="u_att", tag="ua", bufs=4
                )
                nc.scalar.activation(
                    u_att[:],
                    u_ps_of[r][0:1, sl],
                    mybir.ActivationFunctionType.Exp,
                )
                nc.sync.dma_start(
                    out=att_d[b : b + 1, s0 + j * TC : s0 + (j + 1) * TC],
                    in_=u_att[:],
                )

            # --- rowblock 0: the head is DMA-starved (xt+W arrive at queue
            # bandwidth while a vb-major order would consume a k-tile per
            # 216ns). Interleave ALL 8 vb accumulations (8 psum banks)
            # k-major, so each arrived k-tile feeds 8 matmuls (1.73us) and
            # the required delivery rate drops to the achievable ~220 GB/s.
            s_acc_of[0] = tpool.tile([P, RB], f32r, name="s_acc", tag="sa", bufs=2)
            ps0 = {
                vb: pspool.tile([P, RB], f32, name="ps", tag="ps", bufs=8)
                for vb in range(NVB)
            }
            for k in range(NK - 1):
                for vb in range(NVB):
                    nc.tensor.matmul(
                        ps0[vb][:],
                        w_all[:, vb, k],
                        xt_tiles[k // KG][:, k % KG],
                        start=(k == 0),
                        stop=False,
                    )
            for vb in range(NVB):
                nc.tensor.matmul(
                    ps0[vb][:],
                    w_all[:, vb, NK - 1],
                    xt_tiles[(NK - 1) // KG][:, (NK - 1) % KG],
                    start=False,
                    stop=True,
                )
                # tanh emitted right after this vb's stop-matmul so the
                # ACT work overlaps the remaining vbs' final matmuls
                second_stage(0, vb, ps0[vb])
            pending_r = 0
            xt_tiles = xt_next
            xt_next = load_xt(2) if NRB > 2 else None

            for r in range(1, NRB):
                s_acc_of[r] = tpool.tile(
                    [P, RB], f32r, name="s_acc", tag="sa", bufs=2
                )
                for vb in range(NVB):
                    ps = pspool.tile([P, RB], f32, name="ps", tag="ps", bufs=8)
                    if r == NRB - 1 and vb == NVB - 1:
                        # final vb of the final rowblock: run the matmuls
                        # column-split so the first half's psum completes
                        # ~3.5us before the last matmul and its tanh ->
                        # fused-reduce -> exp -> DMA chain overlaps the
                        # second half's matmuls.
                        for j in range(TS):
                            sl = slice(j * TC, (j + 1) * TC)
                            for k in range(NK):
                                nc.tensor.matmul(
                                    ps[:, sl],
                                    w_all[:, vb, k],
                                    xt_tiles[k // KG][:, k % KG, sl],
                                    start=(k == 0),
                                    stop=(k == NK - 1),
                                )
                            second_stage_tail_half(r, vb, ps, j)
                        # epilogue halves AFTER all main matmuls: their PE
                        # ops must not block the in-order PE mid-stream
                        # (half 0's tanh completes during half 1's matmuls)
                        for j in range(TS):
                            epilogue_tail_half(r, j)
                    else:
                        for k in range(NK):
                            nc.tensor.matmul(
                                ps[:],
                                w_all[:, vb, k],
                                xt_tiles[k // KG][:, k % KG],
                                start=(k == 0),
                                stop=(k == NK - 1),
                            )
                        second_stage(r, vb, ps)
                    if pending_r is not None and vb == 1:
                        epilogue(pending_r)
                        pending_r = None
                pending_r = r
                if r + 1 < NRB:
                    xt_tiles = xt_next
                    xt_next = load_xt(r + 2) if r + 2 < NRB else None
            # pending_r == NRB-1 was fully handled by epilogue_tail_half

    _split_multiwait(nc)
    return nc


# ---------------------------------------------------------------------------
# Host entry point
# ---------------------------------------------------------------------------
def kernel(hidden, z, Ww, bw, Wz, bz, Vw, vb, w_a):
    _install_drain_patch()
    from concourse.bass_utils import run_bass_kernel_spmd

    np_main = ml_dtypes.bfloat16 if MAIN_DT == "bf16" else np.float32

    # ---- host-side shard prep ----
    hid_t = np.ascontiguousarray(
        np.asarray(hidden).astype(np_main).transpose(2, 1, 0)
    )  # [H2, B, S]
    z_t = np.ascontiguousarray(
        np.asarray(z).astype(np_main).transpose(2, 1, 0)
    )  # [H2, B, S]

    w_cat = np.concatenate(
        [np.asarray(Ww), np.asarray(Wz)], axis=0
    ).astype(np_main)  # [H, V]
    # reorder so tile (vb) is [P, NK*P] with per-partition-contiguous rows:
    # w_r[vb, p, k*P+q] = W[k*P+p, vb*P+q]
    w_r = np.ascontiguousarray(
        w_cat.reshape(NK, P, NVB, P).transpose(2, 1, 0, 3)
    ).reshape(NVB, P, NK * P)

    bias = (
        np.asarray(bw).astype(np.float64)
        + np.asarray(bz).astype(np.float64)
        + float(np.asarray(w_a)) * ALPHA_S
    ).astype(np.float32)  # [V]
    bct = np.ascontiguousarray(bias.reshape(NVB, P).T)  # [P, NVB]
    vwt = np.ascontiguousarray(
        np.asarray(Vw).astype(np.float32).reshape(NVB, P).T
    )  # [P, NVB]

    in_maps = []
    for c in range(NCORES):
        xt_c = np.empty((H, ROWS), dtype=np_main)
        xt_c[:H2] = hid_t[:, 2 * c : 2 * c + 2, :].reshape(H2, ROWS)
        xt_c[H2:] = z_t[:, 2 * c : 2 * c + 2, :].reshape(H2, ROWS)
        # pack: xt_p[r, p, k, c] = X^T[k*P+p, r*RB+c]
        xt_p = np.ascontiguousarray(
            xt_c.reshape(NK, P, NRB, RB).transpose(2, 1, 0, 3)
        )
        in_maps.append(
            {
                "xt": xt_p,
                "w": w_r,
                "bct": bct,
                "vwt": vwt,
                "vwt7": np.ascontiguousarray(vwt[:, NVB - 1 : NVB]),
                "ones": np.ones((P, 1), dtype=np.float32),
            }
        )

    nc = _build_nc()
    res = run_bass_kernel_spmd(nc, in_maps, list(range(NCORES)))

    out = np.empty((S, B, 1), dtype=np.float32)
    for c in range(NCORES):
        att = res.results[c]["att"]  # [BC, S] = exp(u); normalize here
        for b in range(BC):
            e = att[b].astype(np.float64)
            out[:, 2 * c + b, 0] = (e / e.sum()).astype(np.float32)
    return out

